# revision 28
# baseline (speedup 1.0000x reference)
"""Causal multi-head self-attention with RoPE on 8 Trainium2 NeuronCores.

Problem: B=2, S=2048, D=2048, 16 heads x head_dim 128, causal mask, RoPE.

Sharding (data + tensor parallel per the hint): 8 cores = 2 batch rows x 4
head-groups (4 heads each). Each core computes, for its batch row and its 4
heads: Q/K/V projections, RoPE, causal softmax attention, and the partial
output projection through its head-group's slice of Wo. The host sums the 4
head-group partials per batch row (row-parallel Wo unshard).

On-device layout is fully transposed so no PE transposes are needed:
  - xT [d, s] resident per s-quarter; projections contract over d.
  - QT/KT computed directly as [hd, s] (head-dim on partitions). The head dim
    is host-permuted even/odd -> halves, so RoPE becomes lane-local ops plus
    one small PSUM->SBUF partition-swap DMA. The permutation cancels in Q.K.
  - scores^T [k, q] = KT_slice.T @ QT; exp on ACT (scale folded in); causal
    mask via gpsimd affine_select on diagonal tiles only (multiplicative 0).
  - O^T [hd, q] = V_slice.T @ w^T accumulated over k-tiles; the softmax
    denominator l = sum_k w^T comes from a ones-column matmul into a [1, q]
    psum; 1/l is partition-broadcast and applied to O^T on the DVE.
  - out^T [d, s] = WoT_slice.T @ O^T, DMA'd straight from PSUM to DRAM.

All matmuls run in float32r (full PE rate; ~1e-4 relative rounding).
"""

import math

import numpy as np

import concourse.bass as bass
import concourse.mybir as mybir
import concourse.tile as tile
from concourse import bacc

B = 2
D = 2048
H_LOC = 4  # heads per core
HD = 128  # head dim
QW = 512  # s-quarter width (and matmul moving width)
N_CORES = 8
THETA = 10000.0
F32 = mybir.dt.float32
F32R = mybir.dt.float32r


def build_program(S=2048, repeat=1):
    """Build the per-core SPMD Bass program (all 8 cores run this).

    repeat>1 re-runs the whole computation serially inside one NEFF;
    used only to measure on-device execution time via the wall-time slope.
    """
    nq = S // QW  # number of s-quarters
    dc = D // HD  # contraction chunks
    scale = 1.0 / math.sqrt(HD)

    nc = bacc.Bacc("TRN2", target_bir_lowering=False, debug=False, num_devices=N_CORES)
    xt_d = nc.dram_tensor("xt", [D, S], F32R, kind="ExternalInput").ap()
    wqt_d = nc.dram_tensor("wqt", [D, H_LOC * HD], F32R, kind="ExternalInput").ap()
    wkt_d = nc.dram_tensor("wkt", [D, H_LOC * HD], F32R, kind="ExternalInput").ap()
    wvt_d = nc.dram_tensor("wvt", [D, H_LOC * HD], F32R, kind="ExternalInput").ap()
    wot_d = nc.dram_tensor("wot", [H_LOC * HD, D], F32R, kind="ExternalInput").ap()
    cos2_d = nc.dram_tensor("cos2", [HD, S], F32, kind="ExternalInput").ap()
    sinpm_d = nc.dram_tensor("sinpm", [HD, S], F32, kind="ExternalInput").ap()
    ones_d = nc.dram_tensor("ones", [HD, 1], F32R, kind="ExternalInput").ap()
    outt_d = nc.dram_tensor("outt", [D, S], F32, kind="ExternalOutput").ap()

    with tile.TileContext(nc) as tc:
        with (
            tc.tile_pool(name="const", bufs=1) as constp,
            tc.tile_pool(name="ktv", bufs=1) as ktvp,
            tc.tile_pool(name="xtp", bufs=1) as xtp,
            tc.tile_pool(name="wstream", bufs=3) as wsp,
            tc.tile_pool(name="qtp", bufs=4) as qtp,
            tc.tile_pool(name="rope", bufs=4) as rtp,
            tc.tile_pool(name="wexp", bufs=5) as wep,
            tc.tile_pool(name="otp", bufs=4) as otp,
            tc.tile_pool(name="bcast", bufs=2) as bcp,
            tc.tile_pool(name="outsb", bufs=2) as outsbp,
            tc.tile_pool(name="ps", bufs=8, space="PSUM") as psp,
        ):
            # Tables are first needed by RoPE (after the first projection
            # d-loop); their DMAs are emitted inside the first quarter's
            # V-projection loop so they don't delay the first matmuls.
            cos2 = constp.tile([HD, S], F32, tag="cos2", name="cos2_sb")
            sinpm = constp.tile([HD, S], F32, tag="sinpm", name="sinpm_sb")
            ones = constp.tile([HD, 1], F32R, tag="ones", name="ones_sb")
            tables_loaded = [False]

            def load_tables():
                if not tables_loaded[0]:
                    tables_loaded[0] = True
                    nc.sync.dma_start(cos2[:], cos2_d[:])
                    nc.sync.dma_start(sinpm[:], sinpm_d[:])
                    nc.sync.dma_start(ones[:], ones_d[:])

            kt = [
                ktvp.tile([HD, S], F32R, tag=f"kt{h}", name=f"kt{h}")
                for h in range(H_LOC)
            ]
            vt = [
                ktvp.tile([HD, QW], F32R, tag=f"v{i}", name=f"v{i}")
                for i in range(S // HD)
            ]

            # pair-partner swap: +-16 within each 32-partition quadrant
            SHUF_MASK = [(i + 16) % 32 for i in range(32)]

            def rope(ps, q, out_ap):
                # out = R(pos) * ps, lane-local thanks to the host-side
                # head-dim permutation that places each RoPE pair partner 16
                # partitions away within the same 32-lane quadrant, so the
                # cross-partition move is a single DVE stream_shuffle.
                # sinpm carries the pair sign (-sin even slot, +sin odd slot).
                sl = slice(q * QW, (q + 1) * QW)
                shuf = rtp.tile([HD, QW], F32, tag="shuf", name="shuf", bufs=3)
                nc.vector.stream_shuffle(shuf[:], ps[:], SHUF_MASK)
                ta = rtp.tile([HD, QW], F32, tag="ta", name="ta", bufs=2)
                nc.vector.tensor_mul(ta[:], ps[:], cos2[:, sl])
                tb = rtp.tile([HD, QW], F32, tag="tb", name="tb", bufs=2)
                nc.vector.tensor_mul(tb[:], shuf[:], sinpm[:, sl])
                nc.vector.tensor_add(out_ap, ta[:], tb[:])

            for q in [qq for _ in range(repeat) for qq in range(nq)]:
                sl = slice(q * QW, (q + 1) * QW)
                xt = []

                # --- KT then QT projections (+ RoPE), then V projection.
                # K/Q first so both RoPE chains drain under the V matmuls and
                # attention starts with everything ready. xt DMAs interleave
                # with the KT weight DMAs (first phase to touch them). ---
                qts = []
                for which, (w_d, tag) in enumerate(((wkt_d, "wk"), (wqt_d, "wq"))):
                    pps = [
                        psp.tile([HD, QW], F32, tag="ps", name=f"pps{h}")
                        for h in range(H_LOC)
                    ]
                    for d in range(dc):
                        if which == 0:
                            x_t = xtp.tile([HD, QW], F32R, tag=f"x{d}", name=f"x{d}")
                            nc.sync.dma_start(
                                x_t[:], xt_d[d * HD : (d + 1) * HD, sl]
                            )
                            xt.append(x_t)
                        w_t = wsp.tile([HD, QW], F32R, tag=tag, name=f"{tag}_t")
                        nc.sync.dma_start(w_t[:], w_d[d * HD : (d + 1) * HD, :])
                        for h in range(H_LOC):
                            nc.tensor.matmul(
                                pps[h][:],
                                w_t[:, h * HD : (h + 1) * HD],
                                xt[d][:],
                                start=(d == 0),
                                stop=(d == dc - 1),
                            )
                        if which == 0 and d == 1:
                            load_tables()
                    for h in range(H_LOC):
                        if which == 0:
                            rope(pps[h][:], q, kt[h][:, sl])
                        else:
                            q_sb = qtp.tile([HD, QW], F32R, tag="qt", name=f"qt{h}")
                            rope(pps[h][:], q, q_sb[:])
                            qts.append(q_sb)

                # --- V projection: V[s_tile, e] for this quarter's s-tiles ---
                vps = [
                    psp.tile([HD, QW], F32, tag="ps", name=f"vps{st}")
                    for st in range(4)
                ]
                for d in range(dc):
                    wv_t = wsp.tile([HD, QW], F32R, tag="wv", name="wv_t")
                    nc.sync.dma_start(wv_t[:], wvt_d[d * HD : (d + 1) * HD, :])
                    for st in range(4):
                        nc.tensor.matmul(
                            vps[st][:],
                            xt[d][:, st * HD : (st + 1) * HD],
                            wv_t[:],
                            start=(d == 0),
                            stop=(d == dc - 1),
                        )
                for st in range(4):
                    nc.scalar.copy(vt[q * 4 + st][:], vps[st][:])

                # --- causal attention for this quarter's queries ---
                nk = (q + 1) * 4  # k-tiles in causal range
                ots = []
                for h in range(H_LOC):
                    ot_ps = psp.tile([HD, QW], F32, tag="ps", name="ot_ps")
                    l_ps = psp.tile([1, QW], F32, tag="ps", name="l_ps")

                    def consume(ki, w_t, ot_ps=ot_ps, l_ps=l_ps, h=h, nk=nk):
                        nc.tensor.matmul(
                            ot_ps[:],
                            vt[ki][:, h * HD : (h + 1) * HD],
                            w_t[:],
                            start=(ki == 0),
                            stop=(ki == nk - 1),
                        )
                        nc.tensor.matmul(
                            l_ps[:],
                            ones[:, 0:1],
                            w_t[:],
                            start=(ki == 0),
                            stop=(ki == nk - 1),
                        )

                    pend = []
                    for ki in range(nk):
                        s_ps = psp.tile([HD, QW], F32, tag="ps", name="s_ps")
                        nc.tensor.matmul(
                            s_ps[:],
                            kt[h][:, ki * HD : (ki + 1) * HD],
                            qts[h][:],
                            start=True,
                            stop=True,
                        )
                        w_t = wep.tile([HD, QW], F32R, tag="wexp", name="w_t")
                        nc.scalar.activation(
                            w_t[:],
                            s_ps[:],
                            mybir.ActivationFunctionType.Exp,
                            scale=scale,
                        )
                        if ki >= q * 4:
                            # diagonal tile: zero out w^T where q_glob < k_glob
                            nc.gpsimd.affine_select(
                                out=w_t[:],
                                in_=w_t[:],
                                compare_op=mybir.AluOpType.is_ge,
                                fill=0.0,
                                base=q * QW - ki * HD,
                                pattern=[[1, QW]],
                                channel_multiplier=-1,
                            )
                        pend.append((ki, w_t))
                        if len(pend) >= 3:
                            consume(*pend.pop(0))
                    for p in pend:
                        consume(*p)

                    rc = bcp.tile([1, QW], F32, tag="rc", name="rc")
                    nc.vector.reciprocal(rc[:], l_ps[:])
                    bc = bcp.tile([HD, QW], F32, tag="bc", name="bc")
                    nc.gpsimd.partition_broadcast(bc[:], rc[:])
                    ot_sb = otp.tile([HD, QW], F32R, tag="ot", name=f"ot{h}")
                    nc.vector.tensor_mul(ot_sb[:], ot_ps[:], bc[:])
                    ots.append(ot_sb)

                # --- partial output projection: out^T[d, q] += WoT.T @ O^T ---
                for g in range(4):
                    ops_ = [
                        psp.tile([HD, QW], F32, tag="ps", name=f"ops{dt}")
                        for dt in range(4)
                    ]
                    for h in range(H_LOC):
                        wo_t = wsp.tile([HD, QW], F32R, tag="wo", name="wo_t", bufs=6)
                        nc.sync.dma_start(
                            wo_t[:],
                            wot_d[h * HD : (h + 1) * HD, g * QW : (g + 1) * QW],
                        )
                        for dt in range(4):
                            nc.tensor.matmul(
                                ops_[dt][:],
                                wo_t[:, dt * HD : (dt + 1) * HD],
                                ots[h][:],
                                start=(h == 0),
                                stop=(h == H_LOC - 1),
                            )
                    for dt in range(4):
                        dg = g * 4 + dt
                        o_sb = outsbp.tile([HD, QW], F32, tag="osb", name="o_sb", bufs=4)
                        nc.scalar.copy(o_sb[:], ops_[dt][:])
                        nc.sync.dma_start(
                            outt_d[dg * HD : (dg + 1) * HD, sl], o_sb[:]
                        )
    nc.compile()
    return nc


def prep_inputs(x, token_positions, Wq, Wk, Wv, Wo):
    """Shard + lay out the full inputs into 8 per-core input maps."""
    S = x.shape[1]
    x = np.asarray(x, np.float32)
    pos = np.asarray(token_positions).astype(np.float32)
    k = np.arange(HD // 2, dtype=np.float32)
    inv_freq = (1.0 / (THETA ** (2.0 * k / HD))).astype(np.float32)
    freqs = pos[:, None] * inv_freq[None, :]  # [S, 64]
    cos = np.cos(freqs).T.astype(np.float32)  # [64, S]
    sin = np.sin(freqs).T.astype(np.float32)
    # head-dim permutation chosen so each RoPE pair partner sits +-16
    # partitions away within the same 32-partition quadrant (enables the
    # on-device stream_shuffle). Partition n holds:
    #   g, r = divmod(n, 32); j = 16*g + (r % 16)   (frequency index)
    #   original dim 2j   if r < 16 ("even" slot, rotates with -sin)
    #   original dim 2j+1 otherwise ("odd" slot, rotates with +sin)
    n = np.arange(HD)
    g, r = n // 32, n % 32
    j = 16 * g + (r % 16)
    odd = (r >= 16).astype(np.int64)
    perm = 2 * j + odd
    cos2 = np.ascontiguousarray(cos[j]).astype(np.float32)  # [128, S]
    sinpm = np.ascontiguousarray(np.where(odd[:, None], sin[j], -sin[j])).astype(
        np.float32
    )
    ones = np.ones((HD, 1), np.float32)
    xts = [np.ascontiguousarray(x[b].T) for b in range(B)]

    in_maps = []
    for c in range(N_CORES):
        b, g = c // 4, c % 4
        rows = slice(g * H_LOC * HD, (g + 1) * H_LOC * HD)

        def permT(W):
            Wg = np.asarray(W, np.float32)[rows]  # [512, D]
            Wg = Wg.reshape(H_LOC, HD, D)[:, perm, :].reshape(H_LOC * HD, D)
            return np.ascontiguousarray(Wg.T)  # [D, 512]

        in_maps.append(
            {
                "xt": xts[b],
                "wqt": permT(Wq),
                "wkt": permT(Wk),
                "wvt": np.ascontiguousarray(np.asarray(Wv, np.float32)[rows].T),
                "wot": np.ascontiguousarray(np.asarray(Wo, np.float32)[:, rows].T),
                "cos2": cos2,
                "sinpm": sinpm,
                "ones": ones,
            }
        )
    return in_maps


def combine_outputs(outts):
    """outts: list of 8 per-core outT [D, S] partials -> full [B, S, D]."""
    return np.stack(
        [
            sum(outts[b * 4 : (b + 1) * 4]).T.astype(np.float32)
            for b in range(B)
        ]
    )


_NC = None


def _get_nc():
    global _NC
    if _NC is None:
        _NC = build_program()
    return _NC


def kernel(x, token_positions, Wq, Wk, Wv, Wo):
    from concourse.bass_utils import run_bass_kernel_spmd

    nc = _get_nc()
    in_maps = prep_inputs(x, token_positions, Wq, Wk, Wv, Wo)
    res = run_bass_kernel_spmd(nc, in_maps, core_ids=list(range(N_CORES)))
    return combine_outputs([r["outt"] for r in res.results])
